# revision 4
# baseline (speedup 1.0000x reference)
"""Data-parallel Trainium2 kernel for nn_ChunkedSourceCompressor.

Shards batch B=32 across the 8 NeuronCores (4 batch elements per core),
replicates the small parameter set, and runs the full forward on-device
via the axon PJRT backend (one program per core, no collectives needed —
all reductions are within a batch element, per the sharding hint).
"""

import numpy as np

D = 65
CHUNK = 64
K = 64
SCALE = D ** (-0.5)
N_CORES = 8

_compiled = {}


def _get_devices():
    import jax

    try:
        devs = jax.devices("axon")
    except Exception:
        devs = jax.devices()
    if len(devs) < N_CORES:
        # fall back to whatever is available (e.g. CPU) — still correct
        return None
    return devs[:N_CORES]


def _forward(x, Wq1, bq1, Wq2, bq2, Wk, bk, Wv, bv, cross_q, pos_enc, Wr, br,
             gamma, beta):
    import jax
    import jax.numpy as jnp

    B, N, d = x.shape
    nb = N // CHUNK

    def gelu(v):
        return jax.nn.gelu(v, approximate=False)

    gavg = jnp.mean(x, axis=1)                                        # [B, D]
    q = (gelu(gavg @ Wq1 + bq1) @ Wq2 + bq2).reshape(B, K, d) * SCALE

    keys = gelu(x @ Wk + bk).reshape(B, nb, CHUNK, d) + pos_enc[None]
    values = gelu(x @ Wv + bv).reshape(B, nb, CHUNK, d) + pos_enc[None]

    scores = jnp.einsum('bkd,bncd->bnkc', q, keys) * SCALE            # [B,nb,K,C]
    w = jax.nn.softmax(scores, axis=-1)
    blk = jnp.einsum('bnkc,bncd->bnkd', w, values)                    # [B,nb,K,D]

    # consume blk in native [B,nb,K,D] layout — no transpose materialization
    cs = jnp.einsum('d,bnkd->bkn', cross_q[0], blk) * SCALE           # [B,K,nb]
    cw = jax.nn.softmax(cs, axis=-1)
    compressed = jnp.einsum('bkn,bnkd->bkd', cw, blk)                 # [B,K,D]

    compressed = compressed + (gavg @ Wr + br)[:, None, :]
    mu = jnp.mean(compressed, axis=-1, keepdims=True)
    var = jnp.var(compressed, axis=-1, keepdims=True)
    return (compressed - mu) * jax.lax.rsqrt(var + 1e-5) * gamma + beta


def kernel(**inputs):
    import jax

    x = np.asarray(inputs["x"], dtype=np.float32)
    B = x.shape[0]
    per = B // N_CORES

    param_names = ["Wq1", "bq1", "Wq2", "bq2", "Wk", "bk", "Wv", "bv",
                   "cross_q", "pos_enc", "Wr", "br", "gamma", "beta"]
    params = [np.asarray(inputs[n], dtype=np.float32) for n in param_names]

    devs = _get_devices()
    if devs is None:
        # single-device fallback
        out = np.asarray(jax.jit(_forward)(x, *params))
        return out.astype(np.float32)

    if "fn" not in _compiled:
        _compiled["fn"] = jax.pmap(
            _forward,
            in_axes=(0,) + (None,) * len(params),
            devices=devs,
        )
    fn = _compiled["fn"]

    x_sh = x.reshape(N_CORES, per, *x.shape[1:])
    out = fn(x_sh, *params)                       # [8, per, K, D]
    out = np.asarray(out).reshape(B, K, D).astype(np.float32)
    return out


def device_exec_time(inputs, iters=3):
    """Median on-device execution time with inputs pre-staged on the cores
    (excludes host<->device transfer of x)."""
    import time

    import jax

    x = np.asarray(inputs["x"], dtype=np.float32)
    per = x.shape[0] // N_CORES
    param_names = ["Wq1", "bq1", "Wq2", "bq2", "Wk", "bk", "Wv", "bv",
                   "cross_q", "pos_enc", "Wr", "br", "gamma", "beta"]
    params = [np.asarray(inputs[n], dtype=np.float32) for n in param_names]
    devs = _get_devices()
    if devs is None:
        return float("nan")
    if "fn" not in _compiled:
        _compiled["fn"] = jax.pmap(
            _forward, in_axes=(0,) + (None,) * len(params), devices=devs)
    fn = _compiled["fn"]
    x_sh = jax.device_put_sharded(
        list(x.reshape(N_CORES, per, *x.shape[1:])), devs)
    out = fn(x_sh, *params)
    out.block_until_ready()
    times = []
    for _ in range(iters):
        t0 = time.time()
        out = fn(x_sh, *params)
        out.block_until_ready()
        times.append(time.time() - t0)
    return sorted(times)[len(times) // 2]


# revision 5
# speedup vs baseline: 1.0330x; 1.0330x over previous
"""Data-parallel Trainium2 kernel for nn_ChunkedSourceCompressor.

Shards batch B=32 across the 8 NeuronCores (4 batch elements per core),
replicates the small parameter set, and runs the full forward on-device
via the axon PJRT backend (one program per core, no collectives needed —
all reductions are within a batch element, per the sharding hint).
"""

import numpy as np

D = 65
CHUNK = 64
K = 64
SCALE = D ** (-0.5)
N_CORES = 8

_compiled = {}


def _get_devices():
    import jax

    try:
        devs = jax.devices("axon")
    except Exception:
        devs = jax.devices()
    if len(devs) < N_CORES:
        # fall back to whatever is available (e.g. CPU) — still correct
        return None
    return devs[:N_CORES]


def _forward(x, Wq1, bq1, Wq2, bq2, Wk, bk, Wv, bv, cross_q, pos_enc, Wr, br,
             gamma, beta):
    import jax
    import jax.numpy as jnp

    B, N, d = x.shape
    nb = N // CHUNK

    def gelu(v):
        return jax.nn.gelu(v, approximate=False)

    gavg = jnp.mean(x, axis=1)                                        # [B, D]
    q = (gelu(gavg @ Wq1 + bq1) @ Wq2 + bq2).reshape(B, K, d) * SCALE

    keys = gelu(x @ Wk + bk).reshape(B, nb, CHUNK, d) + pos_enc[None]
    values = gelu(x @ Wv + bv).reshape(B, nb, CHUNK, d) + pos_enc[None]

    scores = jnp.einsum('bkd,bncd->bnkc', q, keys) * SCALE            # [B,nb,K,C]
    w = jax.nn.softmax(scores, axis=-1)
    blk = jnp.einsum('bnkc,bncd->bnkd', w, values)                    # [B,nb,K,D]
    blk = jnp.transpose(blk, (0, 2, 1, 3))                            # [B,K,nb,D]

    cs = jnp.einsum('d,bknd->bkn', cross_q[0], blk) * SCALE           # [B,K,nb]
    cw = jax.nn.softmax(cs, axis=-1)
    compressed = jnp.einsum('bkn,bknd->bkd', cw, blk)                 # [B,K,D]

    compressed = compressed + (gavg @ Wr + br)[:, None, :]
    mu = jnp.mean(compressed, axis=-1, keepdims=True)
    var = jnp.var(compressed, axis=-1, keepdims=True)
    return (compressed - mu) * jax.lax.rsqrt(var + 1e-5) * gamma + beta


def kernel(**inputs):
    import jax

    x = np.asarray(inputs["x"], dtype=np.float32)
    B = x.shape[0]
    per = B // N_CORES

    param_names = ["Wq1", "bq1", "Wq2", "bq2", "Wk", "bk", "Wv", "bv",
                   "cross_q", "pos_enc", "Wr", "br", "gamma", "beta"]
    params = [np.asarray(inputs[n], dtype=np.float32) for n in param_names]

    devs = _get_devices()
    if devs is None:
        # single-device fallback
        out = np.asarray(jax.jit(_forward)(x, *params))
        return out.astype(np.float32)

    if "fn" not in _compiled:
        _compiled["fn"] = jax.pmap(
            _forward,
            in_axes=(0,) + (None,) * len(params),
            devices=devs,
        )
    fn = _compiled["fn"]

    x_sh = x.reshape(N_CORES, per, *x.shape[1:])
    out = fn(x_sh, *params)                       # [8, per, K, D]
    out = np.asarray(out).reshape(B, K, D).astype(np.float32)
    return out


def device_exec_time(inputs, iters=3):
    """Median on-device execution time with inputs pre-staged on the cores
    (excludes host<->device transfer of x)."""
    import time

    import jax

    x = np.asarray(inputs["x"], dtype=np.float32)
    per = x.shape[0] // N_CORES
    param_names = ["Wq1", "bq1", "Wq2", "bq2", "Wk", "bk", "Wv", "bv",
                   "cross_q", "pos_enc", "Wr", "br", "gamma", "beta"]
    params = [np.asarray(inputs[n], dtype=np.float32) for n in param_names]
    devs = _get_devices()
    if devs is None:
        return float("nan")
    if "fn" not in _compiled:
        _compiled["fn"] = jax.pmap(
            _forward, in_axes=(0,) + (None,) * len(params), devices=devs)
    fn = _compiled["fn"]
    x_sh = jax.device_put_sharded(
        list(x.reshape(N_CORES, per, *x.shape[1:])), devs)
    out = fn(x_sh, *params)
    out.block_until_ready()
    times = []
    for _ in range(iters):
        t0 = time.time()
        out = fn(x_sh, *params)
        out.block_until_ready()
        times.append(time.time() - t0)
    return sorted(times)[len(times) // 2]


# revision 6
# speedup vs baseline: 1.1598x; 1.1227x over previous
"""Data-parallel Trainium2 kernel for nn_ChunkedSourceCompressor.

Shards batch B=32 across the 8 NeuronCores (4 batch elements per core),
replicates the small parameter set, and runs the full forward on-device
via the axon PJRT backend (one program per core, no collectives needed —
all reductions are within a batch element, per the sharding hint).
"""

import numpy as np

D = 65
CHUNK = 64
K = 64
SCALE = D ** (-0.5)
N_CORES = 8

_compiled = {}


def _get_devices():
    import jax

    try:
        devs = jax.devices("axon")
    except Exception:
        devs = jax.devices()
    if len(devs) < N_CORES:
        # fall back to whatever is available (e.g. CPU) — still correct
        return None
    return devs[:N_CORES]


def _forward(x, Wq1, bq1, Wq2, bq2, Wk, bk, Wv, bv, cross_q, pos_enc, Wr, br,
             gamma, beta):
    import jax
    import jax.numpy as jnp

    B, N, d = x.shape
    nb = N // CHUNK

    def gelu(v):
        return jax.nn.gelu(v, approximate=False)

    bf16 = jnp.bfloat16
    f32 = jnp.float32

    def mm(spec, a, b):
        # bf16 operands (4x PE column rate vs fp32), fp32 accumulation
        return jnp.einsum(spec, a.astype(bf16), b.astype(bf16),
                          preferred_element_type=f32)

    gavg = jnp.mean(x, axis=1)                                        # [B, D]
    q = (gelu(gavg @ Wq1 + bq1) @ Wq2 + bq2).reshape(B, K, d) * SCALE

    keys = gelu(mm('bnd,de->bne', x, Wk) + bk).reshape(B, nb, CHUNK, d) \
        + pos_enc[None]
    values = gelu(mm('bnd,de->bne', x, Wv) + bv).reshape(B, nb, CHUNK, d) \
        + pos_enc[None]

    scores = mm('bkd,bncd->bnkc', q, keys) * SCALE                    # [B,nb,K,C]
    w = jax.nn.softmax(scores, axis=-1)
    blk = mm('bnkc,bncd->bnkd', w, values)                            # [B,nb,K,D]
    blk = jnp.transpose(blk, (0, 2, 1, 3))                            # [B,K,nb,D]

    cs = jnp.einsum('d,bknd->bkn', cross_q[0], blk) * SCALE           # [B,K,nb]
    cw = jax.nn.softmax(cs, axis=-1)
    compressed = mm('bkn,bknd->bkd', cw, blk)                         # [B,K,D]

    compressed = compressed + (gavg @ Wr + br)[:, None, :]
    mu = jnp.mean(compressed, axis=-1, keepdims=True)
    var = jnp.var(compressed, axis=-1, keepdims=True)
    return (compressed - mu) * jax.lax.rsqrt(var + 1e-5) * gamma + beta


def kernel(**inputs):
    import jax

    x = np.asarray(inputs["x"], dtype=np.float32)
    B = x.shape[0]
    per = B // N_CORES

    param_names = ["Wq1", "bq1", "Wq2", "bq2", "Wk", "bk", "Wv", "bv",
                   "cross_q", "pos_enc", "Wr", "br", "gamma", "beta"]
    params = [np.asarray(inputs[n], dtype=np.float32) for n in param_names]

    devs = _get_devices()
    if devs is None:
        # single-device fallback
        out = np.asarray(jax.jit(_forward)(x, *params))
        return out.astype(np.float32)

    if "fn" not in _compiled:
        _compiled["fn"] = jax.pmap(
            _forward,
            in_axes=(0,) + (None,) * len(params),
            devices=devs,
        )
    fn = _compiled["fn"]

    x_sh = x.reshape(N_CORES, per, *x.shape[1:])
    out = fn(x_sh, *params)                       # [8, per, K, D]
    out = np.asarray(out).reshape(B, K, D).astype(np.float32)
    return out


def device_exec_time(inputs, iters=3):
    """Median on-device execution time with inputs pre-staged on the cores
    (excludes host<->device transfer of x)."""
    import time

    import jax

    x = np.asarray(inputs["x"], dtype=np.float32)
    per = x.shape[0] // N_CORES
    param_names = ["Wq1", "bq1", "Wq2", "bq2", "Wk", "bk", "Wv", "bv",
                   "cross_q", "pos_enc", "Wr", "br", "gamma", "beta"]
    params = [np.asarray(inputs[n], dtype=np.float32) for n in param_names]
    devs = _get_devices()
    if devs is None:
        return float("nan")
    if "fn" not in _compiled:
        _compiled["fn"] = jax.pmap(
            _forward, in_axes=(0,) + (None,) * len(params), devices=devs)
    fn = _compiled["fn"]
    x_sh = jax.device_put_sharded(
        list(x.reshape(N_CORES, per, *x.shape[1:])), devs)
    out = fn(x_sh, *params)
    out.block_until_ready()
    times = []
    for _ in range(iters):
        t0 = time.time()
        out = fn(x_sh, *params)
        out.block_until_ready()
        times.append(time.time() - t0)
    return sorted(times)[len(times) // 2]


# revision 7
# speedup vs baseline: 1.1767x; 1.0146x over previous
"""Data-parallel Trainium2 kernel for nn_ChunkedSourceCompressor.

Shards batch B=32 across the 8 NeuronCores (4 batch elements per core),
replicates the small parameter set, and runs the full forward on-device
via the axon PJRT backend (one program per core, no collectives needed —
all reductions are within a batch element, per the sharding hint).
"""

import numpy as np

D = 65
CHUNK = 64
K = 64
SCALE = D ** (-0.5)
N_CORES = 8

_compiled = {}


def _get_devices():
    import jax

    try:
        devs = jax.devices("axon")
    except Exception:
        devs = jax.devices()
    if len(devs) < N_CORES:
        # fall back to whatever is available (e.g. CPU) — still correct
        return None
    return devs[:N_CORES]


def _forward(x, Wq1, bq1, Wq2, bq2, Wk, bk, Wv, bv, cross_q, pos_enc, Wr, br,
             gamma, beta):
    import jax
    import jax.numpy as jnp

    B, N, d = x.shape
    nb = N // CHUNK

    def gelu(v):
        return jax.nn.gelu(v, approximate=False)

    gavg = jnp.mean(x, axis=1)                                        # [B, D]
    q = (gelu(gavg @ Wq1 + bq1) @ Wq2 + bq2).reshape(B, K, d) * SCALE

    keys = gelu(x @ Wk + bk).reshape(B, nb, CHUNK, d) + pos_enc[None]
    values = gelu(x @ Wv + bv).reshape(B, nb, CHUNK, d) + pos_enc[None]

    scores = jnp.einsum('bkd,bncd->bnkc', q, keys) * SCALE            # [B,nb,K,C]
    w = jax.nn.softmax(scores, axis=-1)
    blk = jnp.einsum('bnkc,bncd->bnkd', w, values)                    # [B,nb,K,D]
    blk = jnp.transpose(blk, (0, 2, 1, 3))                            # [B,K,nb,D]

    cs = jnp.einsum('d,bknd->bkn', cross_q[0], blk) * SCALE           # [B,K,nb]
    cw = jax.nn.softmax(cs, axis=-1)
    compressed = jnp.einsum('bkn,bknd->bkd', cw, blk)                 # [B,K,D]

    compressed = compressed + (gavg @ Wr + br)[:, None, :]
    mu = jnp.mean(compressed, axis=-1, keepdims=True)
    var = jnp.var(compressed, axis=-1, keepdims=True)
    return (compressed - mu) * jax.lax.rsqrt(var + 1e-5) * gamma + beta


def kernel(**inputs):
    import jax

    x = np.asarray(inputs["x"], dtype=np.float32)
    B = x.shape[0]
    per = B // N_CORES

    param_names = ["Wq1", "bq1", "Wq2", "bq2", "Wk", "bk", "Wv", "bv",
                   "cross_q", "pos_enc", "Wr", "br", "gamma", "beta"]
    params = [np.asarray(inputs[n], dtype=np.float32) for n in param_names]

    devs = _get_devices()
    if devs is None:
        # single-device fallback
        out = np.asarray(jax.jit(_forward)(x, *params))
        return out.astype(np.float32)

    if "fn" not in _compiled:
        _compiled["fn"] = jax.pmap(
            _forward,
            in_axes=(0,) + (None,) * len(params),
            devices=devs,
        )
    fn = _compiled["fn"]

    x_sh = x.reshape(N_CORES, per, *x.shape[1:])
    out = fn(x_sh, *params)                       # [8, per, K, D]
    out = np.asarray(out).reshape(B, K, D).astype(np.float32)
    return out


def device_exec_time(inputs, iters=3):
    """Median on-device execution time with inputs pre-staged on the cores
    (excludes host<->device transfer of x)."""
    import time

    import jax

    x = np.asarray(inputs["x"], dtype=np.float32)
    per = x.shape[0] // N_CORES
    param_names = ["Wq1", "bq1", "Wq2", "bq2", "Wk", "bk", "Wv", "bv",
                   "cross_q", "pos_enc", "Wr", "br", "gamma", "beta"]
    params = [np.asarray(inputs[n], dtype=np.float32) for n in param_names]
    devs = _get_devices()
    if devs is None:
        return float("nan")
    if "fn" not in _compiled:
        _compiled["fn"] = jax.pmap(
            _forward, in_axes=(0,) + (None,) * len(params), devices=devs)
    fn = _compiled["fn"]
    x_sh = jax.device_put_sharded(
        list(x.reshape(N_CORES, per, *x.shape[1:])), devs)
    out = fn(x_sh, *params)
    out.block_until_ready()
    times = []
    for _ in range(iters):
        t0 = time.time()
        out = fn(x_sh, *params)
        out.block_until_ready()
        times.append(time.time() - t0)
    return sorted(times)[len(times) // 2]
